# revision 1
# baseline (speedup 1.0000x reference)
"""Trainium2 Bass kernel for nn_Attention_19164144075349.

Additive (Bahdanau-style) attention:
  q = (query @ W_ch + b_ch).reshape(B,O,H,E)
  logits[b,o,m,h] = sum_e w_logit[e] * tanh(context[b,m,e] + q[b,o,h,e]) + b_logit
  probs = softmax(logits / temp, axis=m)
  heads = leaky_relu(einsum(probs, memory), 0.01)
  out = heads.reshape(B,O,H*E) @ W_rh + b_rh

The 67M-element elementwise tanh is replaced by a fitted rank-R
separable expansion
  tanh(c+q) ~= g0*c + g0*q + sum_r s_r * tanh(a_r c + b_r) * tanh(g_r q + d_r)
  (mod an additive function of q, which softmax cancels; the g0*q term
   also cancels; g0*c folds into the exp() bias)
so the (m, oh) logit grid becomes TensorE matmuls over a K = R*E
contraction of small tanh feature maps.  Data-parallel over batch:
8 cores x 4 batches.  tanh/exp/parametric_relu all live in the
`exp_and_others` table set -> one ACT table load.  Matmul operands in
bf16 (fp32 matmul costs 2 PE passes), fp32 accumulation in PSUM.
"""
import numpy as np
import ml_dtypes

import concourse.bass as bass
import concourse.tile as tile
from concourse import bacc, mybir
from concourse.bass_utils import run_bass_kernel_spmd

F32 = mybir.dt.float32
BF16 = mybir.dt.bfloat16
AF = mybir.ActivationFunctionType

B, O, M, H, E = 32, 64, 128, 4, 64
NCORES = 8
BL = B // NCORES
OH = O * H

# --- fitted constants (generated by gen_consts.py; do not edit by hand) ---
# FIT_BEGIN
FIT_R = 12
FIT_A = [np.float64(-1.8486623453214208), np.float64(-1.131106086703141), np.float64(1.236237294436886), np.float64(1.0767697163892294), np.float64(1.4540299501564171), np.float64(-0.9429576428038564), np.float64(-1.0757489449439372), np.float64(1.3941652687432782), np.float64(1.542041183569558), np.float64(1.326207481190978), np.float64(2.0781206607774836), np.float64(0.7006401293553728)]
FIT_B = [np.float64(1.7620790335090903), np.float64(-0.3298550346774668), np.float64(1.6995789522921274), np.float64(-1.4174297402398144), np.float64(0.4019237976761618), np.float64(2.566175721249427), np.float64(-1.9161009461176823), np.float64(2.8626943735274537), np.float64(5.052823891872333), np.float64(-1.9416565973875872), np.float64(-0.8051546866694195), np.float64(-1.1561856982098526)]
FIT_G = [np.float64(-0.6137733927786847), np.float64(1.0907950613693507), np.float64(-1.3255783769344747), np.float64(-1.0960051438957137), np.float64(-1.3408938654086717), np.float64(0.8577698721088615), np.float64(1.0358132956034902), np.float64(-0.7169260714202259), np.float64(0.246113485796956), np.float64(1.2188827449226498), np.float64(-2.053952934763705), np.float64(-1.4940467650164446)]
FIT_D = [np.float64(-0.6525149342569279), np.float64(-0.6069355884963258), np.float64(2.470550577263602), np.float64(-1.4948381026993942), np.float64(1.0734408134159337), np.float64(3.822342171107605), np.float64(-1.892535122286257), np.float64(1.5838174074088518), np.float64(1.965424580869082), np.float64(1.298911102112945), np.float64(0.40201938643519025), np.float64(-2.4489511727618525)]
FIT_S = [np.float64(-0.9818459031866446), np.float64(-2.2433232408969017), np.float64(2.257812375725589), np.float64(-6.112104021995894), np.float64(1.7125861318118099), np.float64(-0.48448015651683557), np.float64(-4.100693129716866), np.float64(2.0095624279497906), np.float64(0.17851262203352217), np.float64(-3.182230171731222), np.float64(0.3455094464747592), np.float64(2.317740001020133)]
FIT_G0 = -0.012953345077302036
# FIT_END
# --------------------------------------------------------------------------

_COMPILED = None


def _build():
    R = FIT_R
    NCH = R // 2
    nc = bacc.Bacc("TRN2", target_bir_lowering=False, debug=False,
                   num_devices=NCORES)

    # packed inputs: one [128,*] bf16 blob, one [65,*] bf16 blob, one fp32 vec blob
    F128 = BL * M + BL * E                    # ctx2 (doubled) + mem
    F65 = BL * O + H * E + H * E + E          # qaT + W_aug + wrh' + brh row
    FV = 5 * NCH + 1                          # av bv gv dv sv + gw col
    d_b128 = nc.dram_tensor("b128", [128, F128], BF16, kind="ExternalInput").ap()
    d_b65 = nc.dram_tensor("b65", [E + 1, F65], BF16, kind="ExternalInput").ap()
    d_vec = nc.dram_tensor("vec", [128, FV], F32, kind="ExternalInput").ap()
    d_out = nc.dram_tensor("out", [O, BL, E], F32, kind="ExternalOutput").ap()

    with tile.TileContext(nc) as tc:
        from contextlib import ExitStack
        with ExitStack() as ctx:
            cons = ctx.enter_context(tc.tile_pool(name="cons", bufs=1))
            feat = ctx.enter_context(tc.tile_pool(name="feat", bufs=1))
            work = ctx.enter_context(tc.tile_pool(name="work", bufs=2))
            psum = ctx.enter_context(tc.tile_pool(name="psum", bufs=8, space="PSUM"))

            vec = cons.tile([128, FV], F32)
            nc.gpsimd.dma_start(vec[:], d_vec)
            b128 = cons.tile([128, F128], BF16)
            for i in range(3):
                c0 = i * 256
                nc.gpsimd.dma_start(b128[:, c0:c0 + 256], d_b128[:, c0:c0 + 256])
            b65 = cons.tile([E + 1, F65], BF16)
            nc.gpsimd.dma_start(b65[:], d_b65)

            # views into the packs
            ctx2 = b128[:, 0:BL * M].rearrange("p (b m) -> p b m", b=BL)
            mem = b128[:, BL * M:BL * M + BL * E].rearrange(
                "p (b e) -> p b e", b=BL)
            qaT = b65[:, 0:BL * O].rearrange("p (b o) -> p b o", b=BL)
            wa = b65[:, BL * O:BL * O + H * E]
            wrh = b65[0:E, BL * O + H * E:BL * O + 2 * H * E].rearrange(
                "p (h e) -> p h e", h=H)
            brh = b65[0:1, BL * O + 2 * H * E:BL * O + 2 * H * E + E]
            av = vec[:, 0 * NCH:1 * NCH]
            bv = vec[:, 1 * NCH:2 * NCH]
            gv = vec[:, 2 * NCH:3 * NCH]
            dv = vec[:, 3 * NCH:4 * NCH]
            sv = vec[:, 4 * NCH:5 * NCH]
            gw = vec[0:E, 5 * NCH:5 * NCH + 1]

            ones_b = cons.tile([128, 128], BF16)
            nc.vector.memset(ones_b[:], 1.0)
            ones_f = cons.tile([128, 128], F32)
            nc.vector.memset(ones_f[:], 1.0)
            gw_b = cons.tile([E, 1], BF16)
            nc.vector.tensor_copy(gw_b[:], gw)

            # trigger the ACT table load immediately (no data deps)
            dummy = work.tile([1, 1], F32, tag="dummy")
            nc.scalar.activation(dummy[:], ones_f[0:1, 0:1], AF.Exp)

            # PE warmup: keep HAM busy so real matmuls run at 2.4 GHz
            warm_ps = psum.tile([64, 64], F32, tag="ps")
            for _ in range(48):
                nc.tensor.matmul(warm_ps[:], lhsT=ones_b[:, 0:64],
                                 rhs=ones_b[:, 0:64], start=True, stop=True)

            # ---- q^T blocks (doubled on partitions via col tile_position) ----
            q2 = cons.tile([128, BL, OH], BF16)
            for b in range(BL):
                qt_ps = psum.tile([128, OH], F32, tag="ps")
                for h in range(H):
                    nc.tensor.matmul(
                        qt_ps[0:64, bass.ts(h, 64)],
                        lhsT=wa[:, bass.ts(h, 64)], rhs=qaT[:, b, :],
                        start=True, stop=True)
                    nc.tensor.matmul(
                        qt_ps[64:128, bass.ts(h, 64)],
                        lhsT=wa[:, bass.ts(h, 64)], rhs=qaT[:, b, :],
                        start=True, stop=True, tile_position=(0, 64))
                nc.vector.tensor_copy(q2[:, b, :], qt_ps[:])

            # ---- tanh feature maps (one chunk = 2 terms; scale/bias per-partition) ----
            fcs, fqs = [], []
            for p in range(NCH):
                fc = feat.tile([128, BL, M], BF16, tag=f"fc{p}")
                nc.scalar.activation(fc[:], ctx2, AF.Tanh,
                                     bias=bv[:, p:p + 1], scale=av[:, p:p + 1])
                nc.vector.tensor_scalar_mul(fc[:], fc[:], sv[:, p:p + 1])
                fcs.append(fc)
            for p in range(NCH):
                fq = feat.tile([128, BL, OH], BF16, tag=f"fq{p}")
                nc.scalar.activation(fq[:], q2[:], AF.Tanh,
                                     bias=dv[:, p:p + 1], scale=gv[:, p:p + 1])
                fqs.append(fq)

            # ---- per-batch pipeline, phase-ordered so ScalarE FIFO
            # (exp/prelu) never serializes across batches ----
            out_all = cons.tile([O, BL, E], F32)
            wc_all = psum.tile([128, BL], F32, tag="ps")
            wc_sbs = []
            for b in range(BL):
                nc.tensor.matmul(wc_all[:, b:b + 1], lhsT=ctx2[0:64, b, :],
                                 rhs=gw_b[:], start=True, stop=True)
                wc_sb = work.tile([128, 1], F32, tag=f"wc{b}")
                nc.vector.tensor_copy(wc_sb[:], wc_all[:, b:b + 1])
                wc_sbs.append(wc_sb)
            log_list, E1_list = [], []
            for b in range(BL):
                # [128, 512] = one full bank: logits in 0:256, sumexp row in 256:512
                log_ps = psum.tile([128, 2 * OH], F32, tag="ps")
                for p in range(NCH):
                    nc.tensor.matmul(log_ps[:, 0:OH], lhsT=fcs[p][:, b, :],
                                     rhs=fqs[p][:, b, :],
                                     start=(p == 0), stop=(p == NCH - 1))
                E1 = work.tile([128, OH], BF16, tag=f"E1{b}")
                nc.scalar.activation(E1[:], log_ps[:, 0:OH], AF.Exp,
                                     bias=wc_sbs[b][:])
                log_list.append(log_ps); E1_list.append(E1)

            luT_list = []
            for b in range(BL):
                log_ps, E1 = log_list[b], E1_list[b]
                se_ps = log_ps[0:1, OH:2 * OH]
                nc.tensor.matmul(se_ps, lhsT=ones_b[:, 0:1], rhs=E1[:],
                                 start=True, stop=True)
                inv_sb = work.tile([1, OH], F32, tag=f"inv{b}")
                nc.vector.reciprocal_approx_fast(out=inv_sb[:], in_=se_ps)
                ib_ps = log_ps[:, 0:OH]
                for chnk in range(2):
                    nc.tensor.matmul(ib_ps[:, bass.ts(chnk, 128)],
                                     lhsT=ones_f[0:1, 0:128],
                                     rhs=inv_sb[0:1, bass.ts(chnk, 128)],
                                     start=True, stop=True)
                probs = work.tile([128, OH], BF16, tag=f"probs{b}")
                nc.vector.tensor_mul(probs[:], E1[:], ib_ps)

                uo_ps = psum.tile([E, OH + E], F32, tag="ps")
                ut_ps = uo_ps[:, 0:OH]
                nc.tensor.matmul(ut_ps, lhsT=mem[:, b, :], rhs=probs[:],
                                 start=True, stop=True)
                luT = work.tile([E, OH], BF16, tag=f"luT{b}")
                nc.scalar.activation(luT[:], ut_ps, AF.Prelu, alpha=0.01)
                luT_list.append((uo_ps, luT))

            for b in range(BL):
                uo_ps, luT = luT_list[b]
                out_ps = uo_ps[:, OH:OH + E]
                for h in range(H):
                    nc.tensor.matmul(out_ps, lhsT=luT[:, bass.ts(h, 64)],
                                     rhs=wrh[:, h, :], start=(h == 0), stop=False)
                nc.tensor.matmul(out_ps, lhsT=ones_b[0:1, 0:64],
                                 rhs=brh, start=False, stop=True)
                nc.vector.tensor_copy(out_all[:, b, :], out_ps)
                if b == 1:
                    nc.gpsimd.dma_start(d_out[:, 0:2, :], out_all[:, 0:2, :])
            nc.gpsimd.dma_start(d_out[:, 2:BL, :], out_all[:, 2:BL, :])

    nc.compile()
    return nc


def _host_prep(query, context, memory, W_ch, b_ch, w_logit, b_logit, W_rh,
               b_rh, temp):
    R = FIT_R
    NCH = R // 2
    bf = ml_dtypes.bfloat16
    a = np.asarray(FIT_A, np.float32)
    bb_ = np.asarray(FIT_B, np.float32)
    g = np.asarray(FIT_G, np.float32)
    d = np.asarray(FIT_D, np.float32)
    s = np.asarray(FIT_S, np.float32)
    g0 = np.float32(FIT_G0)

    inv_temp = np.float32(1.0) / np.float32(temp)
    w_eff = w_logit.astype(np.float32)

    ones64 = np.ones(64, np.float32)
    def dup(x):
        return np.stack([np.concatenate([x[2 * p] * ones64, x[2 * p + 1] * ones64])
                         for p in range(NCH)], axis=1)
    av, bv, gv, dv = dup(a), dup(bb_), dup(g), dup(d)
    sv = np.stack([np.concatenate([s[2 * p] * inv_temp * w_eff,
                                   s[2 * p + 1] * inv_temp * w_eff])
                   for p in range(NCH)], axis=1)
    gwcol = np.zeros((128, 1), np.float32)
    gwcol[0:E, 0] = g0 * inv_temp * w_eff
    vecs = np.concatenate([av, bv, gv, dv, sv, gwcol], axis=1).astype(np.float32)

    wa = np.concatenate([W_ch, b_ch[None, :]], axis=0).astype(np.float32)
    wrh_t = np.ascontiguousarray(
        W_rh.reshape(H, E, E).transpose(1, 0, 2)).reshape(E, H * E)

    F65 = BL * O + H * E + H * E + E
    shard_maps = []
    for c in range(NCORES):
        sl = slice(c * BL, (c + 1) * BL)
        qs = query[sl]
        qaT = np.concatenate([qs.transpose(0, 2, 1),
                              np.ones((BL, 1, O), np.float32)], axis=1)
        qaT = qaT.transpose(1, 0, 2).reshape(E + 1, BL * O)   # [65, BL*O]
        ctxT = context[sl].transpose(2, 0, 1).reshape(E, BL * M)
        ctx2 = np.concatenate([ctxT, ctxT], axis=0)           # [128, BL*M]
        mem = memory[sl].transpose(1, 0, 2).reshape(M, BL * E)
        b128 = np.concatenate([ctx2, mem], axis=1)

        b65 = np.zeros((E + 1, F65), np.float32)
        b65[:, 0:BL * O] = qaT
        b65[:, BL * O:BL * O + H * E] = wa
        b65[0:E, BL * O + H * E:BL * O + 2 * H * E] = wrh_t
        b65[0:1, BL * O + 2 * H * E:BL * O + 2 * H * E + E] = b_rh[None, :]

        shard_maps.append({
            "b128": np.ascontiguousarray(b128).astype(bf),
            "b65": np.ascontiguousarray(b65).astype(bf),
            "vec": np.ascontiguousarray(vecs),
        })
    return shard_maps


def _install_ntff_shim():
    """Provide antenv.axon_hooks (missing on this image) so
    run_bass_kernel_spmd(trace=True) can reach the ctypes NTFF hook."""
    import sys, types
    if "antenv.axon_hooks" in sys.modules:
        return
    mod = types.ModuleType("antenv.axon_hooks")
    mod._hook = None
    def set_axon_ntff_profile_hook(h):
        mod._hook = h
    def get_axon_ntff_profile_hook():
        return mod._hook
    mod.set_axon_ntff_profile_hook = set_axon_ntff_profile_hook
    mod.get_axon_ntff_profile_hook = get_axon_ntff_profile_hook
    sys.modules["antenv.axon_hooks"] = mod
    import antenv
    antenv.axon_hooks = mod
    from trn_agent_boot.trn_boot import _ntff_profile_via_ctypes
    set_axon_ntff_profile_hook(_ntff_profile_via_ctypes("/opt/axon/libaxon_pjrt.so"))
    import concourse.bass_utils as bu
    bu.upload_artifacts = lambda tmpdir: tmpdir


def kernel(trace=False, **inputs):
    global _COMPILED
    if trace:
        try:
            _install_ntff_shim()
        except Exception as e:
            print(f"ntff shim failed: {e}")
    if _COMPILED is None:
        _COMPILED = _build()
    nc = _COMPILED
    shard_maps = _host_prep(**inputs)
    res = run_bass_kernel_spmd(nc, shard_maps, core_ids=list(range(NCORES)),
                               trace=trace)
    out = np.concatenate(
        [res.results[c]["out"].transpose(1, 0, 2) for c in range(NCORES)],
        axis=0).astype(np.float32)
    if trace:
        kernel.last_exec_time_ns = res.exec_time_ns
        kernel.last_results = res
    return out

